# revision 6
# baseline (speedup 1.0000x reference)
"""3-layer GCN (CrystalGCN) on 8 TRN2 NeuronCores — ReduceScatter edition.

Strategy (src-ownership, graph parallel):
  - 50000 nodes -> 6250/core (padded 6272 = 49 tiles of 128). Core c owns
    node range c (both as source rows and as output/dst shard).
  - Edges (EXCLUDING self-loops) assigned to the core owning their SRC:
    all message gathers read LOCAL rows only -> no AllGather of h needed.
  - Per layer: each core scatter-adds its local messages into a partial
    aggregate over ALL destinations (dst-major PSUM: agg[dst,f] +=
    S_j.T @ msgs_j with S_j a [128e x 256dst] one-hot*dinv window built
    on DVE), then one ReduceScatter(add) delivers the dst shard.
  - Self-loop term dinv[i]^2*h[i] is added analytically post-RS.
  - Dense W transform happens on the dst owner after RS (L1/L2); L3 is
    transform-first (tm3 = h2*W3, 16 wide) so its RS payload is tiny.
  - Sliding-window slot schedule: edges sorted by padded dst; 128-edge
    slots each cover a 2-dst-tile window on a static schedule shared by
    all cores (per-core data is packed into the common schedule with
    dummy padding), minimizing slot count vs per-tile quantization.
"""
import numpy as np
import ml_dtypes

N = 50000
E = 800000
F_IN, F_HID, F_OUT = 128, 256, 10
F_OUT_P = 16
NCORES = 8
NSH = N // NCORES            # 6250
P = 128
NT = (NSH + P - 1) // P      # 49 node tiles per core
NSHP = NT * P                # 6272 padded shard rows
NROWS = NSHP * NCORES        # 50176 padded global rows
GT = NROWS // P              # 392 global dst tiles
GB = 8                       # slots per gather call (1024 idxs)

WCOLS = 256 + 512 + 32 + 128   # W1 | W2 | W3 | identity
BF16 = ml_dtypes.bfloat16


def _wrap_idx16(vals):
    """dma_gather index layout: idx i -> [i%16, i//16], replicated to 8
    groups of 16 partitions (one copy per Q7 core)."""
    n = len(vals)
    assert n % 16 == 0
    blk = np.asarray(vals, dtype=np.int16).reshape(n // 16, 16).T
    return np.tile(blk, (8, 1))


def _build_schedule(tile_of_sorted):
    """Common sliding-window slot schedule g[j] (window {g, g+1}) built
    adaptively: each slot's base is the min pending dst-tile over cores,
    so no core's pending edge can ever fall below the window. Returns
    (g, n_slots, per-core slot boundaries)."""
    nc_ = len(tile_of_sorted)
    pos = [0] * nc_
    ne = [len(t) for t in tile_of_sorted]
    g_list = []
    bounds = [[0] for _ in range(nc_)]
    while any(pos[c] < ne[c] for c in range(nc_)):
        gj = min(tile_of_sorted[c][pos[c]]
                 for c in range(nc_) if pos[c] < ne[c])
        gj = int(min(gj, GT - 2))
        for c in range(nc_):
            tl = tile_of_sorted[c]
            end = int(np.searchsorted(tl, gj + 2, side="left"))
            pos[c] = min(pos[c] + P, end)
            bounds[c].append(pos[c])
        g_list.append(gj)
    g = np.array(g_list, dtype=np.int64)
    assert (np.diff(g) >= 0).all()
    return g, len(g_list), [np.array(b, dtype=np.int64) for b in bounds]


def _preprocess(x, edge_index):
    x = np.asarray(x, dtype=np.float32)
    ei = np.asarray(edge_index, dtype=np.int64)
    dst_all = np.concatenate([ei[1], np.arange(N, dtype=np.int64)])
    deg = np.bincount(dst_all, minlength=N).astype(np.float32)
    dinv = np.where(deg > 0, 1.0 / np.sqrt(deg), 0.0).astype(np.float32)

    src, dst = ei[0], ei[1]          # self-loops handled analytically
    core_of = src // NSH
    pd_all = (dst // NSH) * NSHP + (dst % NSH)   # padded global dst row

    per_lr, per_pd, per_tile, per_dst = [], [], [], []
    for c in range(NCORES):
        sel = core_of == c
        lr = (src[sel] % NSH).astype(np.int64)
        pdv = pd_all[sel]
        dstv = dst[sel]
        o = np.argsort(pdv, kind="stable")
        per_lr.append(lr[o])
        per_pd.append(pdv[o])
        per_tile.append((pdv[o] // P).astype(np.int64))
        per_dst.append(dstv[o])

    g, n_slots, bounds = _build_schedule(per_tile)

    idx_streams = np.zeros((NCORES, n_slots * P), dtype=np.int16)
    dslot = np.zeros((NCORES, P, n_slots), dtype=np.float32)
    dinvd = np.zeros((NCORES, P, n_slots), dtype=np.float32)
    lo_act = np.zeros(n_slots, dtype=bool)
    hi_act = np.zeros(n_slots, dtype=bool)
    for c in range(NCORES):
        tl, pdv, lrv, dstv = per_tile[c], per_pd[c], per_lr[c], per_dst[c]
        b = bounds[c]
        for j in range(n_slots):
            s, e = b[j], b[j + 1]
            k = e - s
            if k == 0:
                continue
            idx_streams[c, j * P:j * P + k] = lrv[s:e]
            dslot[c, :k, j] = pdv[s:e] - g[j] * P
            dinvd[c, :k, j] = dinv[dstv[s:e]]
            if (tl[s:e] == g[j]).any():
                lo_act[j] = True
            if (tl[s:e] == g[j] + 1).any():
                hi_act[j] = True

    # per-dst-tile contributor (slot, chunk) lists, common to all cores
    contrib = [[] for _ in range(GT)]
    for j in range(n_slots):
        if hi_act[j]:
            contrib[g[j] + 1].append((j, 1))
        if lo_act[j]:
            contrib[g[j]].append((j, 0))
    for t in range(GT):
        contrib[t].sort(key=lambda jc: jc[0])
        assert contrib[t], f"dst tile {t} has no contributors"

    iota = np.broadcast_to(np.arange(2 * P, dtype=np.float32), (P, 2 * P))

    cores = []
    for c in range(NCORES):
        idx16 = _wrap_idx16(idx_streams[c])
        dinv_node = np.zeros((P, NT), dtype=np.float32)
        loc = np.arange(NSH)
        dinv_node[loc % P, loc // P] = dinv[c * NSH:(c + 1) * NSH]
        meta = np.concatenate(
            [iota, dslot[c], dinvd[c], dinv_node], axis=1).astype(np.float32)
        xls = np.zeros((NSHP, F_IN), dtype=BF16)
        xls[:NSH] = (x[c * NSH:(c + 1) * NSH]
                     * dinv[c * NSH:(c + 1) * NSH, None]).astype(BF16)
        cores.append({"idx16": idx16, "meta": meta, "xls": xls})

    return {"cores": cores, "g": g, "n_slots": n_slots, "contrib": contrib}


def _build_program(n_slots, g, contrib):
    import concourse.bass as bass
    from concourse import bacc
    import concourse.mybir as mybir
    from concourse.tile import TileContext

    dt = mybir.dt
    Alu = mybir.AluOpType
    Act = mybir.ActivationFunctionType

    NS = n_slots
    nc = bacc.Bacc(num_devices=NCORES)
    xls_d = nc.dram_tensor("xls", [NSHP, F_IN], dt.bfloat16, kind="ExternalInput")
    idx_d = nc.dram_tensor("idx16", [P, NS * 8], dt.int16, kind="ExternalInput")
    meta_d = nc.dram_tensor("meta", [P, 2 * P + 2 * NS + NT], dt.float32,
                            kind="ExternalInput")
    wts_d = nc.dram_tensor("wts", [P, WCOLS], dt.bfloat16, kind="ExternalInput")
    bias_d = nc.dram_tensor("bias", [P, 256 + 256 + 16], dt.float32,
                            kind="ExternalInput")
    out_d = nc.dram_tensor("out", [NSHP, F_OUT_P], dt.float32,
                           kind="ExternalOutput")

    n_batches = (NS + GB - 1) // GB
    npairs = GT // 2

    # static close schedule: pair a closes after slot jc[a]
    last_user = np.array([contrib[t][-1][0] for t in range(GT)])
    first_user = np.array([contrib[t][0][0] for t in range(GT)])
    jc = np.maximum(last_user[0::2], last_user[1::2])
    slot_mm = [[] for _ in range(NS)]
    for t in range(GT):
        lst = contrib[t]
        for i, (j, ch) in enumerate(lst):
            slot_mm[j].append((t, ch, i == 0, i == len(lst) - 1))

    with TileContext(nc) as tc:
        with tc.tile_pool(name="const", bufs=1) as cpool, \
             tc.tile_pool(name="msgs", bufs=3) as mpool, \
             tc.tile_pool(name="work", bufs=4) as wpool, \
             tc.tile_pool(name="stage", bufs=2) as spool, \
             tc.tile_pool(name="big", bufs=1) as bigpool, \
             tc.tile_pool(name="ps", bufs=3, space="PSUM") as pspool, \
             tc.tile_pool(name="psd", bufs=2, space="PSUM") as psdpool, \
             tc.tile_pool(name="dram", bufs=1, space="DRAM") as dpool:

            idx_sb = cpool.tile([P, NS * 8], dt.int16)
            nc.sync.dma_start(out=idx_sb[:], in_=idx_d[:])
            meta_sb = cpool.tile([P, 2 * P + 2 * NS + NT], dt.float32)
            nc.sync.dma_start(out=meta_sb[:], in_=meta_d[:])
            wts_sb = cpool.tile([P, WCOLS], dt.bfloat16)
            nc.sync.dma_start(out=wts_sb[:], in_=wts_d[:])
            bias_sb = cpool.tile([P, 256 + 256 + 16], dt.float32)
            nc.sync.dma_start(out=bias_sb[:], in_=bias_d[:])

            iota_ap = meta_sb[:, 0:2 * P]
            dslot0 = 2 * P
            dinvd0 = 2 * P + NS
            dinvn0 = 2 * P + 2 * NS

            agg1_d = dpool.tile([NROWS, F_IN], dt.bfloat16)
            agg1_s = dpool.tile([NSHP, F_IN], dt.bfloat16)
            h1_d = dpool.tile([NSHP, F_HID], dt.bfloat16)
            agg2_d = dpool.tile([NROWS, F_HID], dt.bfloat16)
            agg2_s = dpool.tile([NSHP, F_HID], dt.bfloat16)
            tm3_d = dpool.tile([NSHP, F_IN], dt.bfloat16)
            z3_d = dpool.tile([NROWS, F_OUT_P], dt.float32)
            z3_s = dpool.tile([NSHP, F_OUT_P], dt.float32)

            xls_cache = bigpool.tile([P, NT, F_IN], dt.bfloat16)
            nc.sync.dma_start(
                out=xls_cache[:],
                in_=xls_d[:].rearrange("(t p) f -> p t f", p=P))
            h1_cache = bigpool.tile([P, NT, F_HID], dt.bfloat16)
            tm3_cache = bigpool.tile([P, NT, F_OUT_P], dt.float32)
            tm3_pad = bigpool.tile([P, NT, F_IN], dt.bfloat16)
            nc.vector.memset(tm3_pad[:], 0.0)

            w1_ap = wts_sb[:, 0:256]
            w2_aps = [wts_sb[:, 256:512], wts_sb[:, 512:768]]
            w3_aps = [wts_sb[:, 768:784], wts_sb[:, 784:800]]
            ident_ap = wts_sb[:, 800:928]
            b1_ap = bias_sb[:, 0:256]
            b2_ap = bias_sb[:, 256:512]
            b3_ap = bias_sb[:, 512:528]

            def scatter_layer(l, gsrc, Fg, Fa, agg_dram, out_dt):
                """Gather local msgs (Fg wide), scatter into the partial
                agg (Fa wide) over all dst, stream tiles to agg_dram."""
                psum = {}
                next_a = 0
                stage_t = None
                for b in range(n_batches):
                    nb = min(GB, NS - b * GB)
                    msgs = mpool.tile([P, GB, Fg], dt.bfloat16, tag="msgs",
                                      bufs=3, name=f"msgs_{l}_{b}")
                    nc.gpsimd.dma_gather(
                        out_ap=msgs[:, 0:nb, :], in_ap=gsrc[:, :],
                        idxs_ap=idx_sb[:, b * GB * 8:(b * GB + nb) * 8],
                        num_idxs=nb * P, num_idxs_reg=nb * P,
                        elem_size=Fg)
                    for bi in range(nb):
                        j = b * GB + bi
                        if slot_mm[j]:
                            s_t = wpool.tile(
                                [P, 2 * P], dt.bfloat16, tag="s_t",
                                bufs=6, name=f"s_{l}_{j}")
                            nc.vector.tensor_scalar(
                                out=s_t[:], in0=iota_ap,
                                scalar1=meta_sb[:, dslot0 + j:dslot0 + j + 1],
                                scalar2=meta_sb[:, dinvd0 + j:dinvd0 + j + 1],
                                op0=Alu.is_equal, op1=Alu.mult)
                        for (t, ch, st, sp) in slot_mm[j]:
                            a = t // 2
                            if a not in psum:
                                psum[a] = pspool.tile(
                                    [P, 2, Fa], dt.float32, space="PSUM",
                                    tag="aggps", bufs=3,
                                    name=f"aggps_{l}_{a}")
                            nc.tensor.matmul(
                                psum[a][:, t % 2, :],
                                lhsT=s_t[:, ch * P:(ch + 1) * P],
                                rhs=msgs[:, bi, 0:Fa],
                                start=st, stop=sp)
                        while next_a < npairs and jc[next_a] <= j:
                            a = next_a
                            grp = a // 4
                            if stage_t is None:
                                stage_t = spool.tile(
                                    [P, 8, Fa], out_dt, tag="astage",
                                    bufs=2, name=f"astage_{l}_{grp}")
                            off = (a % 4) * 2
                            nc.scalar.copy(
                                out=stage_t[:, off:off + 2, :],
                                in_=psum[a][:])
                            del psum[a]
                            if a % 4 == 3:
                                t0 = grp * 8
                                nc.sync.dma_start(
                                    out=agg_dram[t0 * P:(t0 + 8) * P, :]
                                    .rearrange("(t p) f -> p t f", p=P),
                                    in_=stage_t[:])
                                stage_t = None
                            next_a += 1
                assert next_a == npairs and stage_t is None and not psum

            # ---------------- Layer 1 ----------------
            scatter_layer(1, xls_d, F_IN, F_IN, agg1_d, dt.bfloat16)
            nc.gpsimd.collective_compute(
                "ReduceScatter", mybir.AluOpType.add,
                replica_groups=[list(range(NCORES))],
                ins=[agg1_d[:].opt()], outs=[agg1_s[:].opt()])

            # dense: h1 = relu(agg1 @ W1 + b1) * dinv
            for t in range(NT):
                aggsb = wpool.tile([P, F_IN], dt.bfloat16, tag="d1in",
                                   bufs=3, name=f"d1in_{t}")
                nc.sync.dma_start(
                    out=aggsb[:], in_=agg1_s[t * P:(t + 1) * P, :])
                selft = wpool.tile([P, F_IN], dt.bfloat16, tag="d1self",
                                   bufs=3, name=f"d1self_{t}")
                nc.vector.tensor_scalar(
                    out=selft[:], in0=xls_cache[:, t, :],
                    scalar1=meta_sb[:, dinvn0 + t:dinvn0 + t + 1],
                    scalar2=None, op0=Alu.mult)
                aggf = wpool.tile([P, F_IN], dt.bfloat16, tag="d1agg",
                                  bufs=3, name=f"d1agg_{t}")
                nc.vector.tensor_tensor(
                    out=aggf[:], in0=aggsb[:], in1=selft[:], op=Alu.add)
                tps = psdpool.tile([P, F_IN], dt.bfloat16, space="PSUM",
                                   tag="tp", bufs=2, name=f"tp1_{t}")
                nc.tensor.transpose(tps[:], aggf[:], ident_ap)
                aggT = wpool.tile([P, F_IN], dt.bfloat16, tag="d1T",
                                  bufs=3, name=f"d1T_{t}")
                nc.scalar.copy(out=aggT[:], in_=tps[:])
                zps = psdpool.tile([P, F_HID], dt.float32, space="PSUM",
                                   tag="z", bufs=2, name=f"z1_{t}")
                nc.tensor.matmul(zps[:], lhsT=aggT[:], rhs=w1_ap,
                                 start=True, stop=True)
                tmp = wpool.tile([P, F_HID], dt.float32, tag="z1t",
                                 bufs=3, name=f"z1t_{t}")
                nc.vector.tensor_tensor(out=tmp[:], in0=zps[:], in1=b1_ap,
                                        op=Alu.add)
                nc.scalar.activation(
                    out=h1_cache[:, t, :], in_=tmp[:], func=Act.Relu,
                    scale=meta_sb[:, dinvn0 + t:dinvn0 + t + 1])
            nc.sync.dma_start(
                out=h1_d[:].rearrange("(t p) f -> p t f", p=P),
                in_=h1_cache[:])

            # ---------------- Layer 2 ----------------
            scatter_layer(2, h1_d, F_HID, F_HID, agg2_d, dt.bfloat16)
            nc.gpsimd.collective_compute(
                "ReduceScatter", mybir.AluOpType.add,
                replica_groups=[list(range(NCORES))],
                ins=[agg2_d[:].opt()], outs=[agg2_s[:].opt()])

            # dense: h2 = relu(agg2 @ W2 + b2) * dinv; tm3 = h2 @ W3
            for t in range(NT):
                aggsb = wpool.tile([P, F_HID], dt.bfloat16, tag="d2in",
                                   bufs=3, name=f"d2in_{t}")
                nc.sync.dma_start(
                    out=aggsb[:], in_=agg2_s[t * P:(t + 1) * P, :])
                selft = wpool.tile([P, F_HID], dt.bfloat16, tag="d2self",
                                   bufs=3, name=f"d2self_{t}")
                nc.vector.tensor_scalar(
                    out=selft[:], in0=h1_cache[:, t, :],
                    scalar1=meta_sb[:, dinvn0 + t:dinvn0 + t + 1],
                    scalar2=None, op0=Alu.mult)
                aggf = wpool.tile([P, F_HID], dt.bfloat16, tag="d2agg",
                                  bufs=3, name=f"d2agg_{t}")
                nc.vector.tensor_tensor(
                    out=aggf[:], in0=aggsb[:], in1=selft[:], op=Alu.add)
                zps = psdpool.tile([P, F_HID], dt.float32, space="PSUM",
                                   tag="z", bufs=2, name=f"z2_{t}")
                for fc in range(2):
                    tps = psdpool.tile([P, P], dt.bfloat16, space="PSUM",
                                       tag="tp", bufs=2,
                                       name=f"tp2_{t}_{fc}")
                    nc.tensor.transpose(
                        tps[:], aggf[:, fc * P:(fc + 1) * P], ident_ap)
                    aggT = wpool.tile([P, P], dt.bfloat16, tag="d2T",
                                      bufs=3, name=f"d2T_{t}_{fc}")
                    nc.scalar.copy(out=aggT[:], in_=tps[:])
                    nc.tensor.matmul(zps[:], lhsT=aggT[:], rhs=w2_aps[fc],
                                     start=(fc == 0), stop=(fc == 1))
                tmp = wpool.tile([P, F_HID], dt.float32, tag="z2t",
                                 bufs=3, name=f"z2t_{t}")
                nc.vector.tensor_tensor(out=tmp[:], in0=zps[:], in1=b2_ap,
                                        op=Alu.add)
                h2t = wpool.tile([P, F_HID], dt.bfloat16, tag="h2",
                                 bufs=3, name=f"h2_{t}")
                nc.scalar.activation(
                    out=h2t[:], in_=tmp[:], func=Act.Relu,
                    scale=meta_sb[:, dinvn0 + t:dinvn0 + t + 1])
                t3ps = psdpool.tile([P, F_OUT_P], dt.float32, space="PSUM",
                                    tag="z", bufs=2, name=f"t3_{t}")
                for fc in range(2):
                    tps = psdpool.tile([P, P], dt.bfloat16, space="PSUM",
                                       tag="tp", bufs=2,
                                       name=f"tp3_{t}_{fc}")
                    nc.tensor.transpose(
                        tps[:], h2t[:, fc * P:(fc + 1) * P], ident_ap)
                    h2T = wpool.tile([P, P], dt.bfloat16, tag="h2T",
                                     bufs=3, name=f"h2T_{t}_{fc}")
                    nc.scalar.copy(out=h2T[:], in_=tps[:])
                    nc.tensor.matmul(t3ps[:], lhsT=h2T[:],
                                     rhs=w3_aps[fc][:, 0:F_OUT_P],
                                     start=(fc == 0), stop=(fc == 1))
                nc.scalar.copy(out=tm3_cache[:, t, :], in_=t3ps[:])
                nc.scalar.copy(out=tm3_pad[:, t, 0:F_OUT_P], in_=t3ps[:])
            nc.sync.dma_start(
                out=tm3_d[:].rearrange("(t p) f -> p t f", p=P),
                in_=tm3_pad[:])

            # ---------------- Layer 3 ----------------
            scatter_layer(3, tm3_d, F_IN, F_OUT_P, z3_d, dt.float32)
            nc.gpsimd.collective_compute(
                "ReduceScatter", mybir.AluOpType.add,
                replica_groups=[list(range(NCORES))],
                ins=[z3_d[:].opt()], outs=[z3_s[:].opt()])

            out_big = bigpool.tile([P, NT, F_OUT_P], dt.float32)
            nc.vector.memset(out_big[:], 0.0)
            for t in range(NT):
                zsb = wpool.tile([P, F_OUT_P], dt.float32, tag="z3in",
                                 bufs=3, name=f"z3in_{t}")
                nc.sync.dma_start(
                    out=zsb[:], in_=z3_s[t * P:(t + 1) * P, :])
                selft = wpool.tile([P, F_OUT_P], dt.float32, tag="z3self",
                                   bufs=3, name=f"z3self_{t}")
                nc.vector.tensor_scalar(
                    out=selft[:], in0=tm3_cache[:, t, :],
                    scalar1=meta_sb[:, dinvn0 + t:dinvn0 + t + 1],
                    scalar2=None, op0=Alu.mult)
                z1 = wpool.tile([P, F_OUT_P], dt.float32, tag="z3a",
                                bufs=3, name=f"z3a_{t}")
                nc.vector.tensor_tensor(
                    out=z1[:], in0=zsb[:], in1=selft[:], op=Alu.add)
                tmp = wpool.tile([P, F_OUT_P], dt.float32, tag="z3b",
                                 bufs=3, name=f"z3b_{t}")
                nc.vector.tensor_tensor(
                    out=tmp[:], in0=z1[:], in1=b3_ap[:, 0:F_OUT_P],
                    op=Alu.add)
                mx = wpool.tile([P, 1], dt.float32, tag="mx", bufs=3,
                                name=f"mx_{t}")
                nc.vector.tensor_reduce(
                    out=mx[:], in_=tmp[:, 0:F_OUT],
                    axis=mybir.AxisListType.X, op=Alu.max, negate=True)
                ex = wpool.tile([P, F_OUT], dt.float32, tag="ex", bufs=3,
                                name=f"ex_{t}")
                nc.scalar.activation(out=ex[:], in_=tmp[:, 0:F_OUT],
                                     func=Act.Exp, bias=mx[:])
                sm = wpool.tile([P, 1], dt.float32, tag="sm", bufs=3,
                                name=f"sm_{t}")
                nc.vector.tensor_reduce(
                    out=sm[:], in_=ex[:], axis=mybir.AxisListType.X,
                    op=Alu.add)
                ls = wpool.tile([P, 1], dt.float32, tag="ls", bufs=3,
                                name=f"ls_{t}")
                nc.scalar.activation(out=ls[:], in_=sm[:], func=Act.Ln)
                nls = wpool.tile([P, 1], dt.float32, tag="nls", bufs=3,
                                 name=f"nls_{t}")
                nc.vector.tensor_scalar(
                    out=nls[:], in0=ls[:], scalar1=-1.0, scalar2=None,
                    op0=Alu.mult)
                nc.vector.tensor_scalar(
                    out=out_big[:, t, 0:F_OUT], in0=tmp[:, 0:F_OUT],
                    scalar1=mx[:], scalar2=nls[:],
                    op0=Alu.add, op1=Alu.add)
            nc.sync.dma_start(
                out=out_d[:].rearrange("(t p) f -> p t f", p=P),
                in_=out_big[:])

    nc.finalize()
    return nc


_CACHE = {}


def kernel(x, edge_index, W1, b1, W2, b2, W3, b3):
    from concourse.bass_utils import run_bass_kernel_spmd

    prep = _preprocess(x, edge_index)
    n_slots = prep["n_slots"]

    key = (n_slots, tuple(int(v) for v in prep["g"][::37]))
    if key not in _CACHE:
        _CACHE[key] = _build_program(n_slots, prep["g"], prep["contrib"])
    nc = _CACHE[key]

    W1 = np.asarray(W1, np.float32)
    W2 = np.asarray(W2, np.float32)
    W3 = np.asarray(W3, np.float32)
    wts = np.zeros((P, WCOLS), dtype=BF16)
    wts[:, 0:256] = W1.astype(BF16)
    wts[:, 256:512] = W2[0:128].astype(BF16)
    wts[:, 512:768] = W2[128:256].astype(BF16)
    wts[:, 768:778] = W3[0:128].astype(BF16)
    wts[:, 784:794] = W3[128:256].astype(BF16)
    wts[:, 800:928] = np.eye(P, dtype=np.float32).astype(BF16)
    bias = np.zeros((P, 256 + 256 + 16), dtype=np.float32)
    bias[:, 0:256] = np.asarray(b1, np.float32)[None, :]
    bias[:, 256:512] = np.asarray(b2, np.float32)[None, :]
    bias[:, 512:522] = np.asarray(b3, np.float32)[None, :]

    in_maps = []
    for c in range(NCORES):
        m = dict(prep["cores"][c])
        m["wts"] = wts
        m["bias"] = bias
        in_maps.append(m)

    res = run_bass_kernel_spmd(nc, in_maps, core_ids=list(range(NCORES)))
    out = np.zeros((N, F_OUT), dtype=np.float32)
    for c in range(NCORES):
        out[c * NSH:(c + 1) * NSH] = res.results[c]["out"][:NSH, :F_OUT]
    return out


# revision 13
# speedup vs baseline: 1.1651x; 1.1651x over previous
"""3-layer GCN (CrystalGCN) on 8 TRN2 NeuronCores — ReduceScatter edition.

Strategy (src-ownership, graph parallel):
  - 50000 nodes -> 6250/core (padded 6272 = 49 tiles of 128). Core c owns
    node range c (both as source rows and as output/dst shard).
  - Edges (EXCLUDING self-loops) assigned to the core owning their SRC:
    all message gathers read LOCAL rows only -> no AllGather of h needed.
  - Per layer: each core scatter-adds its local messages into a partial
    aggregate over ALL destinations (dst-major PSUM: agg[dst,f] +=
    S_j.T @ msgs_j with S_j a [128e x 256dst] one-hot*dinv window built
    on DVE), then one ReduceScatter(add) delivers the dst shard.
  - Self-loop term dinv[i]^2*h[i] is added analytically post-RS.
  - Dense W transform happens on the dst owner after RS (L1/L2); L3 is
    transform-first (tm3 = h2*W3, 16 wide) so its RS payload is tiny.
  - Sliding-window slot schedule: edges sorted by padded dst; 128-edge
    slots each cover a 2-dst-tile window on a static schedule shared by
    all cores (per-core data is packed into the common schedule with
    dummy padding), minimizing slot count vs per-tile quantization.
"""
import numpy as np
import ml_dtypes

N = 50000
E = 800000
F_IN, F_HID, F_OUT = 128, 256, 10
F_OUT_P = 16
NCORES = 8
NSH = N // NCORES            # 6250
P = 128
NT = (NSH + P - 1) // P      # 49 node tiles per core
NSHP = NT * P                # 6272 padded shard rows
NROWS = NSHP * NCORES        # 50176 padded global rows
GT = NROWS // P              # 392 global dst tiles
GB = 8                       # slots per gather call (1024 idxs)

WCOLS = 256 + 512 + 32 + 128 + 256   # W1 | W2 | W3 | identity | iota
BF16 = ml_dtypes.bfloat16


def _wrap_idx16(vals):
    """dma_gather index layout: idx i -> [i%16, i//16], replicated to 8
    groups of 16 partitions (one copy per Q7 core)."""
    n = len(vals)
    assert n % 16 == 0
    blk = np.asarray(vals, dtype=np.int16).reshape(n // 16, 16).T
    return np.tile(blk, (8, 1))


def _build_schedule(tile_of_sorted):
    """Common sliding-window slot schedule g[j] (window {g, g+1}) built
    adaptively: each slot's base is the min pending dst-tile over cores,
    so no core's pending edge can ever fall below the window. Returns
    (g, n_slots, per-core slot boundaries)."""
    nc_ = len(tile_of_sorted)
    pos = [0] * nc_
    ne = [len(t) for t in tile_of_sorted]
    g_list = []
    bounds = [[0] for _ in range(nc_)]
    while any(pos[c] < ne[c] for c in range(nc_)):
        gj = min(tile_of_sorted[c][pos[c]]
                 for c in range(nc_) if pos[c] < ne[c])
        gj = int(min(gj, GT - 2))
        for c in range(nc_):
            tl = tile_of_sorted[c]
            end = int(np.searchsorted(tl, gj + 2, side="left"))
            pos[c] = min(pos[c] + P, end)
            bounds[c].append(pos[c])
        g_list.append(gj)
    g = np.array(g_list, dtype=np.int64)
    assert (np.diff(g) >= 0).all()
    return g, len(g_list), [np.array(b, dtype=np.int64) for b in bounds]


def _preprocess(x, edge_index):
    x = np.asarray(x, dtype=np.float32)
    ei = np.asarray(edge_index, dtype=np.int64)
    dst_all = np.concatenate([ei[1], np.arange(N, dtype=np.int64)])
    deg = np.bincount(dst_all, minlength=N).astype(np.float32)
    dinv = np.where(deg > 0, 1.0 / np.sqrt(deg), 0.0).astype(np.float32)

    src, dst = ei[0], ei[1]          # self-loops handled analytically
    core_of = src // NSH
    pd_all = (dst // NSH) * NSHP + (dst % NSH)   # padded global dst row

    per_lr, per_pd, per_tile, per_dst = [], [], [], []
    for c in range(NCORES):
        sel = core_of == c
        lr = (src[sel] % NSH).astype(np.int64)
        pdv = pd_all[sel]
        dstv = dst[sel]
        o = np.argsort(pdv, kind="stable")
        per_lr.append(lr[o])
        per_pd.append(pdv[o])
        per_tile.append((pdv[o] // P).astype(np.int64))
        per_dst.append(dstv[o])

    g, n_slots, bounds = _build_schedule(per_tile)

    idx_streams = np.zeros((NCORES, n_slots * P), dtype=np.int16)
    dslot = np.zeros((NCORES, P, n_slots), dtype=np.float32)
    dinvd = np.zeros((NCORES, P, n_slots), dtype=np.float32)
    lo_act = np.zeros(n_slots, dtype=bool)
    hi_act = np.zeros(n_slots, dtype=bool)
    for c in range(NCORES):
        tl, pdv, lrv, dstv = per_tile[c], per_pd[c], per_lr[c], per_dst[c]
        b = bounds[c]
        for j in range(n_slots):
            s, e = b[j], b[j + 1]
            k = e - s
            if k == 0:
                continue
            idx_streams[c, j * P:j * P + k] = lrv[s:e]
            dslot[c, :k, j] = pdv[s:e] - g[j] * P
            dinvd[c, :k, j] = dinv[dstv[s:e]]
            if (tl[s:e] == g[j]).any():
                lo_act[j] = True
            if (tl[s:e] == g[j] + 1).any():
                hi_act[j] = True

    # per-dst-tile contributor (slot, chunk) lists, common to all cores
    contrib = [[] for _ in range(GT)]
    for j in range(n_slots):
        if hi_act[j]:
            contrib[g[j] + 1].append((j, 1))
        if lo_act[j]:
            contrib[g[j]].append((j, 0))
    for t in range(GT):
        contrib[t].sort(key=lambda jc: jc[0])
        assert contrib[t], f"dst tile {t} has no contributors"

    cores = []
    for c in range(NCORES):
        idx16 = _wrap_idx16(idx_streams[c])
        dinv_node = np.zeros((P, NT), dtype=np.float32)
        loc = np.arange(NSH)
        dinv_node[loc % P, loc // P] = dinv[c * NSH:(c + 1) * NSH]
        meta = np.concatenate(
            [dslot[c], dinvd[c], dinv_node], axis=1).astype(np.float32)
        xls = np.zeros((NSHP, F_IN), dtype=BF16)
        xls[:NSH] = (x[c * NSH:(c + 1) * NSH]
                     * dinv[c * NSH:(c + 1) * NSH, None]).astype(BF16)
        cores.append({"idx16": idx16, "meta": meta, "xls": xls})

    return {"cores": cores, "g": g, "n_slots": n_slots, "contrib": contrib}


def _build_program(n_slots, g, contrib):
    import concourse.bass as bass
    from concourse import bacc
    import concourse.mybir as mybir
    from concourse.tile import TileContext

    dt = mybir.dt
    Alu = mybir.AluOpType
    Act = mybir.ActivationFunctionType

    NS = n_slots
    nc = bacc.Bacc(num_devices=NCORES)
    xls_d = nc.dram_tensor("xls", [NSHP, F_IN], dt.bfloat16, kind="ExternalInput")
    idx_d = nc.dram_tensor("idx16", [P, NS * 8], dt.int16, kind="ExternalInput")
    meta_d = nc.dram_tensor("meta", [P, 2 * NS + NT], dt.float32,
                            kind="ExternalInput")
    wts_d = nc.dram_tensor("wts", [P, WCOLS], dt.bfloat16, kind="ExternalInput")
    bias_d = nc.dram_tensor("bias", [P, 256 + 256 + 16], dt.float32,
                            kind="ExternalInput")
    out_d = nc.dram_tensor("out", [NSHP, F_OUT_P], dt.float32,
                           kind="ExternalOutput")

    n_batches = (NS + GB - 1) // GB
    npairs = GT // 2

    # static close schedule: pair a closes after slot jc[a]
    last_user = np.array([contrib[t][-1][0] for t in range(GT)])
    first_user = np.array([contrib[t][0][0] for t in range(GT)])
    jc = np.maximum(last_user[0::2], last_user[1::2])
    slot_mm = [[] for _ in range(NS)]
    for t in range(GT):
        lst = contrib[t]
        for i, (j, ch) in enumerate(lst):
            slot_mm[j].append((t, ch, i == 0, i == len(lst) - 1))

    with TileContext(nc) as tc:
        with tc.tile_pool(name="const", bufs=1) as cpool, \
             tc.tile_pool(name="msgs", bufs=3) as mpool, \
             tc.tile_pool(name="work", bufs=4) as wpool, \
             tc.tile_pool(name="stage", bufs=2) as spool, \
             tc.tile_pool(name="big", bufs=1) as bigpool, \
             tc.tile_pool(name="dram", bufs=1, space="DRAM") as dpool:

            idx_sb = cpool.tile([P, NS * 8], dt.int16)
            nc.sync.dma_start(out=idx_sb[:], in_=idx_d[:])
            meta_sb = cpool.tile([P, 2 * NS + NT], dt.float32)
            nc.sync.dma_start(out=meta_sb[:], in_=meta_d[:])
            wts_sb = cpool.tile([P, WCOLS], dt.bfloat16)
            nc.sync.dma_start(out=wts_sb[:], in_=wts_d[:])
            bias_sb = cpool.tile([P, 256 + 256 + 16], dt.float32)
            nc.sync.dma_start(out=bias_sb[:], in_=bias_d[:])

            dslot0 = 0
            dinvd0 = NS
            dinvn0 = 2 * NS

            agg1_d = dpool.tile([NROWS, F_IN], dt.float16)
            agg1_s = dpool.tile([NSHP, F_IN], dt.float16)
            h1_d = dpool.tile([NSHP, F_HID], dt.bfloat16)
            agg2_d = dpool.tile([NROWS, F_HID], dt.float16)
            agg2_s = dpool.tile([NSHP, F_HID], dt.float16)
            tm3_d = dpool.tile([NSHP, F_IN], dt.bfloat16)
            z3_d = dpool.tile([NROWS, F_OUT_P], dt.float32)
            z3_s = dpool.tile([NSHP, F_OUT_P], dt.float32)

            xls_cache = bigpool.tile([P, NT, F_IN], dt.bfloat16)
            nc.sync.dma_start(
                out=xls_cache[:],
                in_=xls_d[:].rearrange("(t p) f -> p t f", p=P))
            h1_cache = bigpool.tile([P, NT, F_HID], dt.bfloat16)
            tm3_cache = bigpool.tile([P, NT, F_OUT_P], dt.float32)
            tm3_pad = bigpool.tile([P, NT, F_IN], dt.bfloat16)
            nc.vector.memset(tm3_pad[:], 0.0)

            w1_ap = wts_sb[:, 0:256]
            w2_aps = [wts_sb[:, 256:512], wts_sb[:, 512:768]]
            w3_aps = [wts_sb[:, 768:784], wts_sb[:, 784:800]]
            ident_ap = wts_sb[:, 800:928]
            iota_ap = wts_sb[:, 928:1184]
            b1_ap = bias_sb[:, 0:256]
            b2_ap = bias_sb[:, 256:512]
            b3_ap = bias_sb[:, 512:528]

            def scatter_layer(l, gsrc, Fg, Fa, agg_dram, out_dt):
                """Gather local msgs (Fg wide), scatter into the partial
                agg (Fa wide) over all dst, stream tiles to agg_dram."""
                with tc.tile_pool(name=f"ps{l}", bufs=4,
                                  space="PSUM") as pspool:
                    psum = {}
                    next_a = 0
                    stage_t = None
                    for b in range(n_batches):
                        nb = min(GB, NS - b * GB)
                        msgs = mpool.tile([P, GB, Fg], dt.bfloat16,
                                          tag="msgs", bufs=3,
                                          name=f"msgs_{l}_{b}")
                        nc.gpsimd.dma_gather(
                            out_ap=msgs[:, 0:nb, :], in_ap=gsrc[:, :],
                            idxs_ap=idx_sb[:, b * GB * 8:(b * GB + nb) * 8],
                            num_idxs=nb * P, num_idxs_reg=nb * P,
                            elem_size=Fg)
                        for bi in range(nb):
                            j = b * GB + bi
                            if slot_mm[j]:
                                s_t = wpool.tile(
                                    [P, 2 * P], dt.bfloat16, tag="s_t",
                                    bufs=6, name=f"s_{l}_{j}")
                                nc.vector.tensor_scalar(
                                    out=s_t[:], in0=iota_ap,
                                    scalar1=meta_sb[:, dslot0 + j:
                                                    dslot0 + j + 1],
                                    scalar2=meta_sb[:, dinvd0 + j:
                                                    dinvd0 + j + 1],
                                    op0=Alu.is_equal, op1=Alu.mult)
                            for (t, ch, st, sp) in slot_mm[j]:
                                a = t // 2
                                if a not in psum:
                                    psum[a] = pspool.tile(
                                        [P, 2, Fa], dt.float32,
                                        space="PSUM", tag="aggps", bufs=4,
                                        name=f"aggps_{l}_{a}")
                                nc.tensor.matmul(
                                    psum[a][:, t % 2, :],
                                    lhsT=s_t[:, ch * P:(ch + 1) * P],
                                    rhs=msgs[:, bi, 0:Fa],
                                    start=st, stop=sp)
                            while next_a < npairs and jc[next_a] <= j:
                                a = next_a
                                grp = a // 4
                                if stage_t is None:
                                    stage_t = spool.tile(
                                        [P, 8, Fa], out_dt, tag="astage",
                                        bufs=2, name=f"astage_{l}_{grp}")
                                off = (a % 4) * 2
                                nc.scalar.copy(
                                    out=stage_t[:, off:off + 2, :],
                                    in_=psum[a][:])
                                del psum[a]
                                if a % 4 == 3:
                                    t0 = grp * 8
                                    nc.sync.dma_start(
                                        out=agg_dram[t0 * P:(t0 + 8) * P, :]
                                        .rearrange("(t p) f -> p t f", p=P),
                                        in_=stage_t[:])
                                    stage_t = None
                                next_a += 1
                    assert next_a == npairs and stage_t is None and not psum

            # ---------------- Layer 1 ----------------
            scatter_layer(1, xls_d, F_IN, F_IN, agg1_d, dt.float16)
            nc.gpsimd.collective_compute(
                "ReduceScatter", mybir.AluOpType.add,
                replica_groups=[list(range(NCORES))],
                ins=[agg1_d[:].opt()], outs=[agg1_s[:].opt()])

            # dense: h1 = relu(agg1 @ W1 + b1) * dinv
            with tc.tile_pool(name="psd1", bufs=3, space="PSUM") as psd:
                for t in range(NT):
                    aggsb = wpool.tile([P, F_IN], dt.float16, tag="d1in",
                                       bufs=4, name=f"d1in_{t}")
                    nc.sync.dma_start(
                        out=aggsb[:], in_=agg1_s[t * P:(t + 1) * P, :])
                    selft = wpool.tile([P, F_IN], dt.bfloat16, tag="d1self",
                                       bufs=4, name=f"d1self_{t}")
                    nc.vector.tensor_scalar(
                        out=selft[:], in0=xls_cache[:, t, :],
                        scalar1=meta_sb[:, dinvn0 + t:dinvn0 + t + 1],
                        scalar2=None, op0=Alu.mult)
                    aggf = wpool.tile([P, F_IN], dt.bfloat16, tag="d1agg",
                                      bufs=4, name=f"d1agg_{t}")
                    nc.vector.tensor_tensor(
                        out=aggf[:], in0=aggsb[:], in1=selft[:], op=Alu.add)
                    tps = psd.tile([P, F_IN], dt.bfloat16, space="PSUM",
                                   tag="tp", bufs=3, name=f"tp1_{t}")
                    nc.tensor.transpose(tps[:], aggf[:], ident_ap)
                    aggT = wpool.tile([P, F_IN], dt.bfloat16, tag="d1T",
                                      bufs=4, name=f"d1T_{t}")
                    nc.scalar.copy(out=aggT[:], in_=tps[:])
                    zps = psd.tile([P, F_HID], dt.float32, space="PSUM",
                                   tag="z", bufs=3, name=f"z1_{t}")
                    nc.tensor.matmul(zps[:], lhsT=aggT[:], rhs=w1_ap,
                                     start=True, stop=True)
                    tmp = wpool.tile([P, F_HID], dt.float32, tag="z1t",
                                     bufs=4, name=f"z1t_{t}")
                    nc.vector.tensor_tensor(out=tmp[:], in0=zps[:],
                                            in1=b1_ap, op=Alu.add)
                    nc.scalar.activation(
                        out=h1_cache[:, t, :], in_=tmp[:], func=Act.Relu,
                        scale=meta_sb[:, dinvn0 + t:dinvn0 + t + 1])
            nc.sync.dma_start(
                out=h1_d[:].rearrange("(t p) f -> p t f", p=P),
                in_=h1_cache[:])

            # ---------------- Layer 2 ----------------
            scatter_layer(2, h1_d, F_HID, F_HID, agg2_d, dt.float16)
            nc.gpsimd.collective_compute(
                "ReduceScatter", mybir.AluOpType.add,
                replica_groups=[list(range(NCORES))],
                ins=[agg2_d[:].opt()], outs=[agg2_s[:].opt()])

            # dense: h2 = relu(agg2 @ W2 + b2) * dinv; tm3 = h2 @ W3
            with tc.tile_pool(name="psd2", bufs=3, space="PSUM") as psd:
                for t in range(NT):
                    aggsb = wpool.tile([P, F_HID], dt.float16, tag="d2in",
                                       bufs=4, name=f"d2in_{t}")
                    nc.sync.dma_start(
                        out=aggsb[:], in_=agg2_s[t * P:(t + 1) * P, :])
                    selft = wpool.tile([P, F_HID], dt.bfloat16,
                                       tag="d2self", bufs=4,
                                       name=f"d2self_{t}")
                    nc.vector.tensor_scalar(
                        out=selft[:], in0=h1_cache[:, t, :],
                        scalar1=meta_sb[:, dinvn0 + t:dinvn0 + t + 1],
                        scalar2=None, op0=Alu.mult)
                    aggf = wpool.tile([P, F_HID], dt.bfloat16, tag="d2agg",
                                      bufs=4, name=f"d2agg_{t}")
                    nc.vector.tensor_tensor(
                        out=aggf[:], in0=aggsb[:], in1=selft[:], op=Alu.add)
                    tps = psd.tile([P, 2, P], dt.bfloat16, space="PSUM",
                                   tag="tp", bufs=3, name=f"tp2_{t}")
                    for fc in range(2):
                        nc.tensor.transpose(
                            tps[:, fc, :], aggf[:, fc * P:(fc + 1) * P],
                            ident_ap)
                    aggT = wpool.tile([P, 2, P], dt.bfloat16, tag="d2T",
                                      bufs=4, name=f"d2T_{t}")
                    nc.scalar.copy(out=aggT[:], in_=tps[:])
                    zps = psd.tile([P, F_HID], dt.float32, space="PSUM",
                                   tag="z", bufs=3, name=f"z2_{t}")
                    for fc in range(2):
                        nc.tensor.matmul(zps[:], lhsT=aggT[:, fc, :],
                                         rhs=w2_aps[fc],
                                         start=(fc == 0), stop=(fc == 1))
                    tmp = wpool.tile([P, F_HID], dt.float32, tag="z2t",
                                     bufs=4, name=f"z2t_{t}")
                    nc.vector.tensor_tensor(out=tmp[:], in0=zps[:],
                                            in1=b2_ap, op=Alu.add)
                    h2t = wpool.tile([P, F_HID], dt.bfloat16, tag="h2",
                                     bufs=4, name=f"h2_{t}")
                    nc.scalar.activation(
                        out=h2t[:], in_=tmp[:], func=Act.Relu,
                        scale=meta_sb[:, dinvn0 + t:dinvn0 + t + 1])
                    tps3 = psd.tile([P, 2, P], dt.bfloat16, space="PSUM",
                                    tag="tp", bufs=3, name=f"tp3_{t}")
                    for fc in range(2):
                        nc.tensor.transpose(
                            tps3[:, fc, :], h2t[:, fc * P:(fc + 1) * P],
                            ident_ap)
                    h2T = wpool.tile([P, 2, P], dt.bfloat16, tag="h2T",
                                     bufs=4, name=f"h2T_{t}")
                    nc.vector.tensor_scalar(
                        out=h2T[:], in0=tps3[:], scalar1=1.0,
                        scalar2=None, op0=Alu.mult)
                    t3ps = psd.tile([P, F_OUT_P], dt.float32, space="PSUM",
                                    tag="z", bufs=3, name=f"t3_{t}")
                    for fc in range(2):
                        nc.tensor.matmul(t3ps[:], lhsT=h2T[:, fc, :],
                                         rhs=w3_aps[fc][:, 0:F_OUT_P],
                                         start=(fc == 0), stop=(fc == 1))
                    nc.scalar.copy(out=tm3_cache[:, t, :], in_=t3ps[:])
                    nc.scalar.copy(out=tm3_pad[:, t, 0:F_OUT_P],
                                   in_=t3ps[:])
            nc.sync.dma_start(
                out=tm3_d[:].rearrange("(t p) f -> p t f", p=P),
                in_=tm3_pad[:])

            # ---------------- Layer 3 ----------------
            scatter_layer(3, tm3_d, F_IN, F_OUT_P, z3_d, dt.float32)
            nc.gpsimd.collective_compute(
                "ReduceScatter", mybir.AluOpType.add,
                replica_groups=[list(range(NCORES))],
                ins=[z3_d[:].opt()], outs=[z3_s[:].opt()])

            # batched log_softmax: group Act functions to avoid per-tile
            # activation-table reloads (Exp/Ln each loaded once).
            out_big = bigpool.tile([P, NT, F_OUT_P], dt.float32)
            nc.vector.memset(out_big[:], 0.0)
            tmp_all = bigpool.tile([P, NT, F_OUT_P], dt.float32)
            ex_all = bigpool.tile([P, NT, F_OUT], dt.float32)
            mx_all = bigpool.tile([P, NT], dt.float32)
            sm_all = bigpool.tile([P, NT], dt.float32)
            ls_all = bigpool.tile([P, NT], dt.float32)
            nls_all = bigpool.tile([P, NT], dt.float32)
            for t in range(NT):
                zsb = wpool.tile([P, F_OUT_P], dt.float32, tag="z3in",
                                 bufs=4, name=f"z3in_{t}")
                nc.sync.dma_start(
                    out=zsb[:], in_=z3_s[t * P:(t + 1) * P, :])
                selft = wpool.tile([P, F_OUT_P], dt.float32, tag="z3self",
                                   bufs=4, name=f"z3self_{t}")
                nc.vector.tensor_scalar(
                    out=selft[:], in0=tm3_cache[:, t, :],
                    scalar1=meta_sb[:, dinvn0 + t:dinvn0 + t + 1],
                    scalar2=None, op0=Alu.mult)
                z1 = wpool.tile([P, F_OUT_P], dt.float32, tag="z3a",
                                bufs=4, name=f"z3a_{t}")
                nc.vector.tensor_tensor(
                    out=z1[:], in0=zsb[:], in1=selft[:], op=Alu.add)
                nc.vector.tensor_tensor(
                    out=tmp_all[:, t, :], in0=z1[:],
                    in1=b3_ap[:, 0:F_OUT_P], op=Alu.add)
                nc.vector.tensor_reduce(
                    out=mx_all[:, t:t + 1], in_=tmp_all[:, t, 0:F_OUT],
                    axis=mybir.AxisListType.X, op=Alu.max, negate=True)
            for t in range(NT):
                nc.scalar.activation(
                    out=ex_all[:, t, :], in_=tmp_all[:, t, 0:F_OUT],
                    func=Act.Exp, bias=mx_all[:, t:t + 1])
            for t in range(NT):
                nc.vector.tensor_reduce(
                    out=sm_all[:, t:t + 1], in_=ex_all[:, t, :],
                    axis=mybir.AxisListType.X, op=Alu.add)
            nc.scalar.activation(out=ls_all[:], in_=sm_all[:], func=Act.Ln)
            nc.vector.tensor_scalar(
                out=nls_all[:], in0=ls_all[:], scalar1=-1.0, scalar2=None,
                op0=Alu.mult)
            for t in range(NT):
                nc.vector.tensor_scalar(
                    out=out_big[:, t, 0:F_OUT], in0=tmp_all[:, t, 0:F_OUT],
                    scalar1=mx_all[:, t:t + 1],
                    scalar2=nls_all[:, t:t + 1],
                    op0=Alu.add, op1=Alu.add)
            nc.sync.dma_start(
                out=out_d[:].rearrange("(t p) f -> p t f", p=P),
                in_=out_big[:])

    nc.finalize()
    return nc


_CACHE = {}


def kernel(x, edge_index, W1, b1, W2, b2, W3, b3):
    from concourse.bass_utils import run_bass_kernel_spmd

    prep = _preprocess(x, edge_index)
    n_slots = prep["n_slots"]

    key = (n_slots, tuple(int(v) for v in prep["g"][::37]))
    if key not in _CACHE:
        _CACHE[key] = _build_program(n_slots, prep["g"], prep["contrib"])
    nc = _CACHE[key]

    W1 = np.asarray(W1, np.float32)
    W2 = np.asarray(W2, np.float32)
    W3 = np.asarray(W3, np.float32)
    wts = np.zeros((P, WCOLS), dtype=BF16)
    wts[:, 0:256] = W1.astype(BF16)
    wts[:, 256:512] = W2[0:128].astype(BF16)
    wts[:, 512:768] = W2[128:256].astype(BF16)
    wts[:, 768:778] = W3[0:128].astype(BF16)
    wts[:, 784:794] = W3[128:256].astype(BF16)
    wts[:, 800:928] = np.eye(P, dtype=np.float32).astype(BF16)
    wts[:, 928:1184] = np.broadcast_to(
        np.arange(256, dtype=np.float32), (P, 256)).astype(BF16)
    bias = np.zeros((P, 256 + 256 + 16), dtype=np.float32)
    bias[:, 0:256] = np.asarray(b1, np.float32)[None, :]
    bias[:, 256:512] = np.asarray(b2, np.float32)[None, :]
    bias[:, 512:522] = np.asarray(b3, np.float32)[None, :]

    in_maps = []
    for c in range(NCORES):
        m = dict(prep["cores"][c])
        m["wts"] = wts
        m["bias"] = bias
        in_maps.append(m)

    res = run_bass_kernel_spmd(nc, in_maps, core_ids=list(range(NCORES)))
    out = np.zeros((N, F_OUT), dtype=np.float32)
    for c in range(NCORES):
        out[c * NSH:(c + 1) * NSH] = res.results[c]["out"][:NSH, :F_OUT]
    return out
